# revision 1
# baseline (speedup 1.0000x reference)
"""Bipartite graph-attention layer on 8 Trainium2 NeuronCores (Bass/Tile).

Strategy (collective-free SPMD):
  - Edges are sorted by source node on the host; core c owns src rows
    [c*SRC_PC, (c+1)*SRC_PC) and all edges of those rows.
  - Every core builds the full target tables from a host-transposed tgtT
    with one fused matmul per 128 rows (rhs = [W_tgt | W_tgt@a2 | 0...]):
    h_tab[t] = h_tgt(t) (128 f32) and sc_tab[t] = [score_j(t) | pad] (64 f32).
  - score_i for the local src shard via tiny matvec matmuls -> si_tab.
  - Edge phase: per batch of GB blocks, dma_gather pulls h rows and score
    rows by tgt id (int16 indices, so the table is processed in quarters
    of <=25000 rows) and score_i rows by src id; per 128-edge tile an
    is_equal one-hot [edge, src] matrix (VectorE) is the stationary matmul
    operand reducing p_e * h_row (and p_e) into per-block PSUM; the
    epilogue normalizes and applies ELU.
No cross-core communication; the host concatenates the 8 output shards.
"""

import math
import sys
from contextlib import ExitStack

import numpy as np

sys.path.insert(0, "/opt/trn_rl_repo")

import concourse.bacc as bacc  # noqa: E402
import concourse.bass as bass  # noqa: E402,F401
import concourse.tile as tile  # noqa: E402
from concourse import mybir  # noqa: E402
from concourse.bass_utils import run_bass_kernel_spmd  # noqa: E402

P = 128
IN = 256
OUT = 128
N_CORES = 8
SLOPE = 0.2
CLIP = 30.0
EPS = 1e-8
QS = 25000  # tgt rows per gather quarter (int16 index limit)
PHASE1 = True  # dev instrumentation: emit phase 1
PHASE2 = True  # dev instrumentation: emit phase 2
P2_REPEAT = 1  # dev instrumentation: repeat phase-2 body
P1_LEVEL = 3  # 0=loads 1=+matmul 2=+copy 3=+table writes
P2_LEVEL = 3  # 0=gathers 1=+p/onehot/Hs 2=+matmuls 3=full epilogue
GB = 8  # blocks per phase-2 batch
SCW = 64  # score-table row width (f32) -> 256B rows
F32 = mybir.dt.float32
I16 = mybir.dt.int16
AF = mybir.ActivationFunctionType
OP = mybir.AluOpType


# ---------------------------------------------------------------- host prep
def _host_prep(src, tgt, W_src, W_tgt, a, edge_index, n_cores):
    n_src = src.shape[0]
    n_tgt = tgt.shape[0]
    src_pc = n_src // n_cores
    n_blocks = (src_pc + P - 1) // P
    nq = (n_tgt + QS - 1) // QS

    s_id = np.asarray(edge_index[0], dtype=np.int64)
    t_id = np.asarray(edge_index[1], dtype=np.int64)
    order = np.argsort(s_id, kind="stable")
    s_s = s_id[order]
    t_s = t_id[order]

    bounds = np.searchsorted(s_s, np.arange(n_cores + 1) * src_pc)
    cores = []
    tq = 1
    for c in range(n_cores):
        lo, hi = int(bounds[c]), int(bounds[c + 1])
        sl = s_s[lo:hi] - c * src_pc
        tl = t_s[lo:hi]
        blk = sl // P
        qv = tl // QS
        key = blk * nq + qv
        o2 = np.argsort(key, kind="stable")
        sl, tl, blk, qv, key = sl[o2], tl[o2], blk[o2], qv[o2], key[o2]
        cnt = np.bincount(key, minlength=n_blocks * nq)
        if len(sl):
            tq = max(tq, int(math.ceil(cnt.max() / P)))
        cores.append((sl, tl, blk, qv, key, cnt))

    ntb = nq * tq
    nt = n_blocks * ntb
    gb = max(1, min(GB, 64 // ntb))
    tgtT = np.ascontiguousarray(np.asarray(tgt, np.float32).T)
    srcT = np.ascontiguousarray(np.asarray(src, np.float32).T)
    w_tgt = np.ascontiguousarray(np.asarray(W_tgt, np.float32))
    w_tgtT = np.ascontiguousarray(w_tgt.T)
    w_srcT = np.ascontiguousarray(np.asarray(W_src, np.float32).T)
    a_np = np.asarray(a, np.float32).reshape(2 * OUT, 1)
    a1 = np.ascontiguousarray(a_np[:OUT])
    a2 = np.ascontiguousarray(a_np[OUT:])
    iota = np.tile(np.arange(P, dtype=np.float32), (P, 1))

    in_maps = []
    for c in range(n_cores):
        sl, tl, blk, qv, key, cnt = cores[c]
        sbk = np.full((P, nt), -1.0, np.float32)
        tgw16 = np.zeros((16, nq * n_blocks * tq * 8), np.int16)
        siw16 = np.zeros((16, nt * 8), np.int16)
        if len(sl):
            off = np.concatenate([[0], np.cumsum(cnt)])
            pos = np.arange(len(sl)) - off[key]
            t_i = pos // P
            p_i = pos % P
            # block-major slot column (for sbk / one-hot / compute order)
            gcol = blk * ntb + qv * tq + t_i
            sbk[p_i, gcol] = (sl - blk * P).astype(np.float32)
            # H/SC gather index arrays: per quarter segment, (block, tile)
            # order, wrapped into 16 rows x 8 col-groups
            hc = qv * (n_blocks * tq * 8) + (blk * tq + t_i) * 8 + p_i // 16
            tgw16[p_i % 16, hc] = (tl - qv * QS).astype(np.int16)
            # SI gather index array: per batch, (quarter, block, tile) order
            bat = blk // gb
            b0 = bat * gb
            nb = np.minimum(gb, n_blocks - b0)
            j = (qv * nb + (blk - b0)) * tq + t_i
            sc_ = b0 * ntb * 8 + j * 8 + p_i // 16
            siw16[p_i % 16, sc_] = ((sl % P) * n_blocks + sl // P).astype(
                np.int16
            )
        in_maps.append(
            {
                "tgtT": tgtT,
                "srcT": np.ascontiguousarray(
                    srcT[:, c * src_pc : (c + 1) * src_pc]
                ),
                "w_tgt": w_tgt,
                "w_tgtT": w_tgtT,
                "w_srcT": w_srcT,
                "a1": a1,
                "a2": a2,
                "iota": iota,
                "tgw": np.tile(tgw16, (8, 1)),
                "siw": np.tile(siw16, (8, 1)),
                "sblk": sbk,
            }
        )
    cfg = dict(
        n_tgt=n_tgt,
        src_pc=src_pc,
        n_blocks=n_blocks,
        tq=tq,
        nq=nq,
        gb=gb,
        n_cores=n_cores,
    )
    return cfg, in_maps


# ---------------------------------------------------------------- device IR
def build_nc(n_tgt, src_pc, n_blocks, tq, nq, gb=GB, n_cores=N_CORES):
    nc = bacc.Bacc(target_bir_lowering=False)
    ntb = nq * tq
    nt = n_blocks * ntb
    src_pad = n_blocks * P
    HW2 = OUT + SCW  # phase-1 psum/staging width

    tgtT = nc.declare_dram_parameter("tgtT", [IN, n_tgt], F32, isOutput=False)
    srcT = nc.declare_dram_parameter("srcT", [IN, src_pc], F32, isOutput=False)
    w_tgt = nc.declare_dram_parameter("w_tgt", [IN, OUT], F32, isOutput=False)
    w_tgtT = nc.declare_dram_parameter("w_tgtT", [OUT, IN], F32, isOutput=False)
    w_srcT = nc.declare_dram_parameter("w_srcT", [OUT, IN], F32, isOutput=False)
    a1 = nc.declare_dram_parameter("a1", [OUT, 1], F32, isOutput=False)
    a2 = nc.declare_dram_parameter("a2", [OUT, 1], F32, isOutput=False)
    iota = nc.declare_dram_parameter("iota", [P, P], F32, isOutput=False)
    tgw = nc.declare_dram_parameter(
        "tgw", [P, nq * n_blocks * tq * 8], I16, isOutput=False
    )
    siw = nc.declare_dram_parameter("siw", [P, nt * 8], I16, isOutput=False)
    sblk = nc.declare_dram_parameter("sblk", [P, nt], F32, isOutput=False)
    out = nc.declare_dram_parameter("out", [src_pad, OUT], F32, isOutput=True)

    h_tab = nc.dram_tensor("h_tab", [n_tgt, OUT], F32)
    sc_tab = nc.dram_tensor("sc_tab", [n_tgt, SCW], F32)
    si_tab = nc.dram_tensor("si_tab", [src_pad, SCW], F32)

    RCH = 2048  # rows per phase-1 DMA chunk

    with tile.TileContext(nc) as tc, ExitStack() as ctx:
        const = ctx.enter_context(tc.tile_pool(name="const", bufs=1))
        psmall = ctx.enter_context(tc.tile_pool(name="psmall", bufs=2, space="PSUM"))

        # ---- constants
        sb_a1 = const.tile([P, 1], F32)
        nc.sync.dma_start(out=sb_a1, in_=a1[:, :])
        sb_a2 = const.tile([P, 1], F32)
        nc.sync.dma_start(out=sb_a2, in_=a2[:, :])
        sb_wtT = const.tile([P, IN], F32)
        nc.sync.dma_start(out=sb_wtT, in_=w_tgtT[:, :])
        sb_wsT = const.tile([P, IN], F32)
        nc.sync.dma_start(out=sb_wsT, in_=w_srcT[:, :])
        sb_iota = const.tile([P, P], F32)
        nc.sync.dma_start(out=sb_iota, in_=iota[:, :])

        # W_aug[kc] = [W_tgt[kc] | w2_chunk | zeros]  -> rhs width HW2
        sb_waug = const.tile([P, 2, HW2], F32)
        nc.vector.memset(sb_waug, 0.0)
        sb_w1 = const.tile([P, 2, 1], F32)
        for kc in range(2):
            nc.sync.dma_start(
                out=sb_waug[:, kc, 0:OUT], in_=w_tgt[kc * P : (kc + 1) * P, :]
            )
            ps_w2 = psmall.tile([P, 1], F32, tag="ps_small")
            nc.tensor.matmul(
                out=ps_w2,
                lhsT=sb_wtT[:, kc * P : (kc + 1) * P],
                rhs=sb_a2,
                start=True,
                stop=True,
            )
            nc.scalar.copy(sb_waug[:, kc, OUT : OUT + 1], ps_w2)
            ps_w1 = psmall.tile([P, 1], F32, tag="ps_small")
            nc.tensor.matmul(
                out=ps_w1,
                lhsT=sb_wsT[:, kc * P : (kc + 1) * P],
                rhs=sb_a1,
                start=True,
                stop=True,
            )
            nc.scalar.copy(sb_w1[:, kc, :], ps_w1)

        si_stage = const.tile([P, n_blocks], F32)
        nc.vector.memset(si_stage, 0.0)

        with tc.tile_pool(name="ld", bufs=2) as ld, tc.tile_pool(
            name="hstage", bufs=4
        ) as hstage, tc.tile_pool(name="psumh", bufs=2, space="PSUM") as psumh:
            # ---- phase 1a: score_i for the local src shard
            blk = 0
            for r0 in (range(0, src_pc, RCH) if PHASE1 else []):
                rch = min(RCH, src_pc - r0)
                xa = ld.tile([P, RCH], F32, tag="xa")
                nc.sync.dma_start(out=xa[:, 0:rch], in_=srcT[0:P, r0 : r0 + rch])
                xb = ld.tile([P, RCH], F32, tag="xb")
                nc.sync.dma_start(out=xb[:, 0:rch], in_=srcT[P:IN, r0 : r0 + rch])
                for s0 in range(0, rch, P):
                    rows = min(P, rch - s0)
                    ps = psmall.tile([P, 1], F32, tag="ps_small")
                    nc.tensor.matmul(
                        out=ps[0:rows, :],
                        lhsT=xa[:, s0 : s0 + rows],
                        rhs=sb_w1[:, 0, :],
                        start=True,
                        stop=False,
                    )
                    nc.tensor.matmul(
                        out=ps[0:rows, :],
                        lhsT=xb[:, s0 : s0 + rows],
                        rhs=sb_w1[:, 1, :],
                        start=False,
                        stop=True,
                    )
                    nc.scalar.copy(si_stage[0:rows, blk : blk + 1], ps[0:rows, :])
                    blk += 1
            assert (blk == n_blocks) or not PHASE1
            si_view = si_tab[:, :].rearrange("(p b) c -> p b c", p=P)
            nc.sync.dma_start(out=si_view[:, :, 0:1], in_=si_stage)
            zpad = ld.tile([P, n_blocks * (SCW - 1)], F32, tag="zpad")
            nc.vector.memset(zpad, 0.0)
            nc.sync.dma_start(out=si_view[:, :, 1:SCW], in_=zpad)

            # ---- phase 1b: h table + score_j table (full, per core)
            for r0 in (range(0, n_tgt, RCH) if PHASE1 else []):
                rch = min(RCH, n_tgt - r0)
                ya = ld.tile([P, RCH], F32, tag="ya")
                nc.sync.dma_start(out=ya[:, 0:rch], in_=tgtT[0:P, r0 : r0 + rch])
                yb = ld.tile([P, RCH], F32, tag="yb")
                nc.sync.dma_start(out=yb[:, 0:rch], in_=tgtT[P:IN, r0 : r0 + rch])
                for s0 in range(0, rch, P):
                    rows = min(P, rch - s0)
                    if P1_LEVEL < 1:
                        continue
                    ps = psumh.tile([P, HW2], F32)
                    nc.tensor.matmul(
                        out=ps[0:rows, :],
                        lhsT=ya[:, s0 : s0 + rows],
                        rhs=sb_waug[:, 0, :],
                        start=True,
                        stop=False,
                    )
                    nc.tensor.matmul(
                        out=ps[0:rows, :],
                        lhsT=yb[:, s0 : s0 + rows],
                        rhs=sb_waug[:, 1, :],
                        start=False,
                        stop=True,
                    )
                    if P1_LEVEL < 2:
                        continue
                    hs = hstage.tile([P, HW2], F32)
                    nc.scalar.copy(hs[0:rows, :], ps[0:rows, :])
                    if P1_LEVEL < 3:
                        continue
                    nc.sync.dma_start(
                        out=h_tab[r0 + s0 : r0 + s0 + rows, :],
                        in_=hs[0:rows, 0:OUT],
                    )
                    nc.sync.dma_start(
                        out=sc_tab[r0 + s0 : r0 + s0 + rows, :],
                        in_=hs[0:rows, OUT:HW2],
                    )

        tc.strict_bb_all_engine_barrier()

        # ---- phase 2: edge processing
        ipool = ctx.enter_context(tc.tile_pool(name="ipool", bufs=2))
        gpool = ctx.enter_context(tc.tile_pool(name="gpool", bufs=2))
        scpool = ctx.enter_context(tc.tile_pool(name="scpool", bufs=2))
        sipool = ctx.enter_context(tc.tile_pool(name="sipool", bufs=2))
        spool = ctx.enter_context(tc.tile_pool(name="spool", bufs=2))
        wpool = ctx.enter_context(tc.tile_pool(name="wpool", bufs=2))
        ohpool = ctx.enter_context(tc.tile_pool(name="ohpool", bufs=3))
        hspool = ctx.enter_context(tc.tile_pool(name="hspool", bufs=3))
        epool = ctx.enter_context(tc.tile_pool(name="epool", bufs=2))
        opool = ctx.enter_context(tc.tile_pool(name="opool", bufs=3))
        psumo = ctx.enter_context(tc.tile_pool(name="psumo", bufs=2, space="PSUM"))

        CAPB = gb * tq  # max (block,tile) slots per quarter per batch
        p2_batches = []
        for _rep in range(P2_REPEAT if PHASE2 else 0):
            p2_batches.extend(range(0, n_blocks, gb))
        for b0 in p2_batches:
            nb = min(gb, n_blocks - b0)
            nbt = nb * tq
            sbt = spool.tile([P, gb * ntb], F32, tag="sbt")
            nc.sync.dma_start(
                out=sbt[:, 0 : nb * ntb], in_=sblk[:, b0 * ntb : (b0 + nb) * ntb]
            )

            Gs, SCs, Ps = [], [], []
            for q in range(nq):
                qlo = q * QS
                qhi = min(n_tgt, qlo + QS)
                ixq = ipool.tile([P, CAPB * 8], I16, tag=f"ix{q}")
                nc.sync.dma_start(
                    out=ixq[:, 0 : nbt * 8],
                    in_=tgw[
                        :,
                        (q * n_blocks + b0) * tq * 8 : (q * n_blocks + b0 + nb)
                        * tq
                        * 8,
                    ],
                )
                Gq = gpool.tile([P, CAPB, OUT], F32, tag=f"G{q}")
                nc.gpsimd.dma_gather(
                    out_ap=Gq[:, 0:nbt, :],
                    in_ap=h_tab[qlo:qhi, :],
                    idxs_ap=ixq[:, 0 : nbt * 8],
                    num_idxs=nbt * P,
                    num_idxs_reg=nbt * P,
                    elem_size=OUT,
                    single_packet=False,
                )
                SCq = scpool.tile([P, CAPB, SCW], F32, tag=f"SC{q}")
                nc.gpsimd.dma_gather(
                    out_ap=SCq[:, 0:nbt, :],
                    in_ap=sc_tab[qlo:qhi, :],
                    idxs_ap=ixq[:, 0 : nbt * 8],
                    num_idxs=nbt * P,
                    num_idxs_reg=nbt * P,
                    elem_size=SCW,
                    single_packet=False,
                )
                Gs.append(Gq)
                SCs.append(SCq)

            six = ipool.tile([P, gb * ntb * 8], I16, tag="six")
            nc.sync.dma_start(
                out=six[:, 0 : nb * ntb * 8],
                in_=siw[:, b0 * ntb * 8 : (b0 + nb) * ntb * 8],
            )
            SIt = sipool.tile([P, gb * ntb, SCW], F32, tag="SIt")
            nc.gpsimd.dma_gather(
                out_ap=SIt[:, 0 : nb * ntb, :],
                in_ap=si_tab[:, :],
                idxs_ap=six[:, 0 : nb * ntb * 8],
                num_idxs=nb * ntb * P,
                num_idxs_reg=nb * ntb * P,
                elem_size=SCW,
                single_packet=False,
            )

            if P2_LEVEL < 1:
                continue
            # p_e per quarter: exp(clip(leaky_relu(si + sj)))
            for q in range(nq):
                x = wpool.tile([P, CAPB], F32, tag="x")
                nc.vector.tensor_tensor(
                    out=x[:, 0:nbt],
                    in0=SIt[:, q * nbt : (q + 1) * nbt, 0],
                    in1=SCs[q][:, 0:nbt, 0],
                    op=OP.add,
                )
                y = wpool.tile([P, CAPB], F32, tag="y")
                nc.vector.tensor_scalar(
                    out=y[:, 0:nbt], in0=x[:, 0:nbt], scalar1=SLOPE,
                    scalar2=None, op0=OP.mult,
                )
                y2 = wpool.tile([P, CAPB], F32, tag="y2")
                nc.vector.tensor_tensor(
                    out=y2[:, 0:nbt], in0=y[:, 0:nbt], in1=x[:, 0:nbt], op=OP.max
                )
                y3 = wpool.tile([P, CAPB], F32, tag="y3")
                nc.vector.tensor_scalar(
                    out=y3[:, 0:nbt], in0=y2[:, 0:nbt], scalar1=CLIP,
                    scalar2=-CLIP, op0=OP.min, op1=OP.max,
                )
                pq = wpool.tile([P, CAPB], F32, tag=f"p{q}")
                nc.scalar.activation(pq[:, 0:nbt], y3[:, 0:nbt], AF.Exp)
                Ps.append(pq)

            for bb in range(nb):
                b = b0 + bb
                ps = psumo.tile([P, OUT], F32, tag="ps_num")
                psd = psumo.tile([P, 1], F32, tag="ps_den")
                for q in range(nq):
                    for t in range(tq):
                        gcol = bb * ntb + q * tq + t
                        qcol = bb * tq + t
                        first = q == 0 and t == 0
                        last = q == nq - 1 and t == tq - 1
                        oh = ohpool.tile([P, P], F32)
                        nc.vector.tensor_tensor(
                            out=oh,
                            in0=sbt[:, gcol : gcol + 1].to_broadcast([P, P]),
                            in1=sb_iota,
                            op=OP.is_equal,
                        )
                        hsc = hspool.tile([P, OUT], F32)
                        nc.scalar.activation(
                            hsc,
                            Gs[q][:, qcol, :],
                            AF.Copy,
                            scale=Ps[q][:, qcol : qcol + 1],
                        )
                        if P2_LEVEL < 2:
                            continue
                        nc.tensor.matmul(
                            out=ps[:, :], lhsT=oh, rhs=hsc, start=first, stop=last
                        )
                        nc.tensor.matmul(
                            out=psd[:, :],
                            lhsT=oh,
                            rhs=Ps[q][:, qcol : qcol + 1],
                            start=first,
                            stop=last,
                        )

                if P2_LEVEL < 3:
                    continue
                # normalize + ELU
                d = epool.tile([P, 1], F32, tag="d")
                nc.vector.tensor_scalar(
                    out=d, in0=psd[:, :], scalar1=EPS, scalar2=None, op0=OP.add
                )
                r = epool.tile([P, 1], F32, tag="r")
                nc.vector.reciprocal(r, d)
                o = epool.tile([P, OUT], F32, tag="o")
                nc.vector.tensor_scalar(
                    out=o, in0=ps[:, :], scalar1=r, scalar2=None, op0=OP.mult
                )
                t0 = epool.tile([P, OUT], F32, tag="t0")
                nc.vector.tensor_scalar(
                    out=t0, in0=o, scalar1=0.0, scalar2=None, op0=OP.min
                )
                u = epool.tile([P, OUT], F32, tag="u")
                nc.scalar.activation(u, t0, AF.Exp)
                v = epool.tile([P, OUT], F32, tag="v")
                nc.vector.tensor_scalar(
                    out=v, in0=u, scalar1=1.0, scalar2=0.0, op0=OP.subtract,
                    op1=OP.min,
                )
                w = epool.tile([P, OUT], F32, tag="w")
                nc.vector.tensor_scalar(
                    out=w, in0=o, scalar1=0.0, scalar2=None, op0=OP.max
                )
                fin = opool.tile([P, OUT], F32)
                nc.vector.tensor_tensor(out=fin, in0=v, in1=w, op=OP.add)
                nc.sync.dma_start(out=out[b * P : (b + 1) * P, :], in_=fin)

    if not nc.is_finalized():
        nc.finalize()
    return nc


# ---------------------------------------------------------------- entry
_CACHE = {}


def run(src, tgt, W_src, W_tgt, a, edge_index, n_cores=N_CORES, trace=False):
    cfg, in_maps = _host_prep(src, tgt, W_src, W_tgt, a, edge_index, n_cores)
    key = tuple(sorted(cfg.items()))
    if key not in _CACHE:
        _CACHE[key] = build_nc(
            n_tgt=cfg["n_tgt"],
            src_pc=cfg["src_pc"],
            n_blocks=cfg["n_blocks"],
            tq=cfg["tq"],
            nq=cfg["nq"],
            gb=cfg["gb"],
            n_cores=cfg["n_cores"],
        )
    nc = _CACHE[key]
    res = run_bass_kernel_spmd(nc, in_maps, list(range(n_cores)), trace=trace)
    outs = [res.results[c]["out"][: cfg["src_pc"]] for c in range(n_cores)]
    return np.concatenate(outs, axis=0).astype(np.float32), res


def kernel(src, tgt, W_src, W_tgt, a, edge_index):
    out, _ = run(src, tgt, W_src, W_tgt, a, edge_index)
    return out

